# revision 9
# baseline (speedup 1.0000x reference)
"""Binarized 3x3 conv (BinaryConnect) on 8 Trainium2 NeuronCores.

Problem: y = conv2d(x, sign(w), stride=1, pad=1) + bias
  x: (32, 256, 56, 56) f32, w: (256, 256, 3, 3) f32, bias: (256,) f32
  out: (32, 256, 56, 56) f32

Strategy (data-parallel over batch, 4 images/core), F(4,3) Winograd
along H + direct along W, all-fp16 datapath (same PE rate as bf16,
8x less quantization error, which the Winograd cancellations need):

  - Host: binarize weights, transform along r with G (F(4,3)),
    cast x to fp16, zero-pad each 56x56 plane into the pitch-57
    layout (shared pad column) used by the baseline.
  - Device, per image/channel-group: DVE+GpSimd compute the 6-point
    B^T forward transform along H into U planes [14 hb x 57] (one
    plane per Winograd point u; W-taps stay direct so every matmul
    rhs is a contiguous shift of a U plane). Per (kg, u, chunk):
    6 matmuls (3 s-taps x 2 cg) of 399 cols accumulate M_u in PSUM;
    the 6 u-points are processed in halves of 3 so at most 6+2 PSUM
    banks are live. Inverse A^T transform runs on DVE/GpSimd
    (scalar_tensor_tensor for the x2/x4/x8 terms), scalar engine
    applies bias + crops the pitch column + interleaves the 4
    Winograd output rows into packed 56x56 planes, DMA out.
  - PE work halves vs direct conv: 6 pts x 3 taps vs 9 taps x
    K=128-pairs => 229,824 vs 459,648 matmul columns per core.
"""

import numpy as np

import concourse.bacc as bacc
import concourse.mybir as mybir
from concourse.tile import TileContext
from concourse.bass_utils import run_bass_kernel_spmd

# problem constants (hardcoded per harness contract)
N_IMG = 32
C = 256  # input channels
K = 256  # output channels
H = W = 56
HP = 58  # padded rows (1 top + 56 + 1 bottom)
WP = 57  # row pitch: 1 shared pad column + 56 data
N_CORES = 8
IMG_PER_CORE = N_IMG // N_CORES

L_PLANE = HP * WP  # 3306
L_PAD = L_PLANE + 4  # 3310
LEAD = 2  # leading slack so tap offset (-1) stays in-bounds
X_SLACK = 188  # tail slack so the strided d_i views can be constructed
HB = 14  # h-blocks of 4 output rows
CHUNK = 7 * WP  # 399 cols per PSUM chunk (2 chunks per plane)
UPL = HB * WP  # 798: one U plane
U_LEAD = 2
UL = U_LEAD + UPL + 4  # 804 pitch between U planes
VL = 800  # pitch between v planes in the vstage tile
NW = 36  # distinct (u, s, cg) weight tiles

FP16 = mybir.dt.float16
F32 = mybir.dt.float32

# F(4,3) weight transform (Lavin / wincnn convention)
G_MAT = np.array(
    [
        [1 / 4, 0, 0],
        [-1 / 6, -1 / 6, -1 / 6],
        [-1 / 6, 1 / 6, -1 / 6],
        [1 / 24, 1 / 12, 1 / 6],
        [1 / 24, -1 / 12, 1 / 6],
        [0, 0, 1],
    ],
    np.float64,
)

_compiled = {}


def _ldw_key(inst):
    ap = inst.ins[0]
    bap = getattr(ap, "bass_ap", None)
    if bap is not None:
        try:
            return (bap.tensor.name, bap.offset, str(bap.ap), str(ap.dtype))
        except AttributeError:
            return None
    try:
        return (ap.memref, ap.offset, str(ap.ap), str(ap.dtype))
    except AttributeError:
        return None


def _dedup_ldweights(ordered):
    """Drop InstLdweights that reload weights already resident in the PE
    array. Only drops clean instructions: no sync_info and sync-deps
    covered by the retained load."""
    n_drop = 0
    for bb, insts in ordered.items():
        out = []
        last_key = None
        last_deps = None
        for inst in insts:
            if isinstance(inst, mybir.InstLdweights):
                key = _ldw_key(inst)
                si = inst.sync_info
                clean = si is None or (not si.on_wait and not si.on_update)
                sdeps = set(inst.sync_dependency_names())
                nsdeps = set(inst.nosync_dependency_names())
                if (
                    key is not None
                    and key == last_key
                    and clean
                    and last_deps is not None
                    and sdeps <= last_deps
                    and not nsdeps
                ):
                    n_drop += 1
                    continue
                last_key = key
                last_deps = sdeps
            elif isinstance(inst, mybir.InstMatmult):
                pass  # does not clobber the weight array
            elif getattr(inst, "engine", None) == mybir.EngineType.PE:
                last_key = None
            out.append(inst)
        ordered[bb] = out
    return n_drop


def _build_bass(loops=1):
    import concourse.tile as tile_mod

    nc = bacc.Bacc()

    xp = nc.declare_dram_parameter("xp", [IMG_PER_CORE, 2, 128, L_PAD], FP16, isOutput=False)
    wt = nc.declare_dram_parameter("wt", [128, NW * 256], FP16, isOutput=False)
    bi = nc.declare_dram_parameter("bi", [2, 128, 1], F32, isOutput=False)
    y = nc.declare_dram_parameter("y", [IMG_PER_CORE, 2, 128, H * W], F32, isOutput=True)

    orig_legalize = tile_mod.tile_legalize

    def legalize_and_dedup(ordered, nc_arg):
        ordered = orig_legalize(ordered, nc_arg)
        _dedup_ldweights(ordered)
        return ordered

    tile_mod.tile_legalize = legalize_and_dedup
    try:
        _build_tile_program(nc, loops, xp, wt, bi, y)
    finally:
        tile_mod.tile_legalize = orig_legalize
    nc.compile()
    return nc


# phase-major x layout: padded plane rows stored as [rows 0,4,..,56 |
# 1,5,..,57 | 2,6,..,54 | 3,7,..,55], so the rows {4*hb+i : hb} that feed
# Winograd point views are flat contiguous 798-col slices.
_PHASE_BASE = (0, 15 * WP, 30 * WP, 44 * WP)  # block starts (15,15,14,14 rows)
_DOFF = (  # offset of d_i = rows 4*hb+i, hb=0..13
    _PHASE_BASE[0],
    _PHASE_BASE[1],
    _PHASE_BASE[2],
    _PHASE_BASE[3],
    _PHASE_BASE[0] + WP,
    _PHASE_BASE[1] + WP,
)


def _dview(xs, i):
    """x rows 4*hb + i (padded index) as a flat [128, 798] view."""
    return xs[:, LEAD + _DOFF[i] : LEAD + _DOFF[i] + UPL]


def _uview(us, u):
    return us[:, u * UL + U_LEAD : u * UL + U_LEAD + UPL]


def _fwd_transform(nc, xs, us, ts):
    """U_u = B^T d along H. DVE: u0,u3,u4,u5 (8 ops); GpSimd: u1,u2 (6 ops)."""
    A = mybir.AluOpType
    d = [_dview(xs, i) for i in range(6)]
    U = [_uview(us, u) for u in range(6)]
    T0 = ts[:, 0:UPL]
    T1 = ts[:, VL : VL + UPL]
    v, g = nc.vector, nc.gpsimd
    # u0 = -5*d2 + (4*d0 + d4)
    v.scalar_tensor_tensor(T0, d[0], 4.0, d[4], A.mult, A.add)
    v.scalar_tensor_tensor(U[0], d[2], -5.0, T0, A.mult, A.add)
    # u1 = -4*(d1+d2) + (d3+d4)
    g.tensor_add(U[1], d[1], d[2])
    g.tensor_add(T1, d[3], d[4])
    v.scalar_tensor_tensor(U[1], U[1], -4.0, T1, A.mult, A.add)
    # u2 = 4*(d1-d2) + (d4-d3)
    g.tensor_sub(U[2], d[1], d[2])
    g.tensor_sub(T1, d[4], d[3])
    v.scalar_tensor_tensor(U[2], U[2], 4.0, T1, A.mult, A.add)
    # u3 = 2*(d3-d1) + (d4-d2);  u4 = -2*(d3-d1) + (d4-d2)
    v.tensor_sub(U[3], d[3], d[1])
    v.tensor_sub(T0, d[4], d[2])
    v.scalar_tensor_tensor(U[4], U[3], -2.0, T0, A.mult, A.add)
    v.scalar_tensor_tensor(U[3], U[3], 2.0, T0, A.mult, A.add)
    # u5 = -5*d3 + (4*d1 + d5)
    v.scalar_tensor_tensor(T0, d[1], 4.0, d[5], A.mult, A.add)
    v.scalar_tensor_tensor(U[5], d[3], -5.0, T0, A.mult, A.add)


def _conv_all(nc, pspool, it, vpool, opool, usb, wtile, bsb, y, kg):
    """u-major GEMMs over all 4 images + incremental inverse transform.

    Per u-step: 6 weight loads (s, cg), each feeding 8 matmuls (4 img x
    2 chunks) into 8 PSUM banks; consumers drain the banks before the
    next u-step needs them. The A^T inverse is applied incrementally:
      u0: m0 = M0        u1: m1 = M1        u2: a = m1+M2, b = m1-M2
      u3: m3 = M3        u4: c = m3+M4, d = m3-M4 (c,d reuse m1,m3 slots)
      u5: v0 = m0+a+c, v1 = 2d+b, v2 = 4c+a, v3 = 8d+b+M5
    (tensor-tensor ops may read only one PSUM input -> M1/M3 staged by
    the scalar engine).
    """
    A = mybir.AluOpType
    M0, M1C, AA, BB, M3D = 0, 1, 2, 3, 4

    def iv(j, n, ch):
        o = ((j * 4 + n) * 2 + ch) * 400
        return it[:, o : o + CHUNK]

    vts = {}
    ps = {}
    for u in range(6):
        for n in range(IMG_PER_CORE):
            for ch in range(2):
                ps[n, ch] = pspool.tile(
                    [128, CHUNK], F32, tag="ps", name=f"ps_{kg}_{u}_{n}_{ch}"
                )
        for si, (s, cgi) in enumerate((s, c) for s in range(3) for c in range(2)):
            col = ((u * 3 + s) * 2 + cgi) * 256 + kg * 128
            wsl = wtile[:, col : col + 128]
            for n in range(IMG_PER_CORE):
                for ch in range(2):
                    base = u * UL + U_LEAD + ch * CHUNK + s - 1
                    nc.tensor.matmul(
                        ps[n, ch][:],
                        lhsT=wsl,
                        rhs=usb[n, cgi][:, base : base + CHUNK],
                        start=(si == 0),
                        stop=(si == 5),
                    )
        for n in range(IMG_PER_CORE):
            for ch in range(2):
                m = ps[n, ch][:]
                if u == 0:
                    nc.scalar.copy(iv(M0, n, ch), m)
                elif u == 1:
                    nc.scalar.copy(iv(M1C, n, ch), m)
                elif u == 2:
                    nc.vector.tensor_add(iv(AA, n, ch), iv(M1C, n, ch), m)
                    nc.vector.tensor_sub(iv(BB, n, ch), iv(M1C, n, ch), m)
                elif u == 3:
                    nc.scalar.copy(iv(M3D, n, ch), m)
                elif u == 4:
                    # c into the (dead) m1 slot, then d in-place over m3
                    nc.vector.tensor_add(iv(M1C, n, ch), iv(M3D, n, ch), m)
                    nc.vector.tensor_sub(iv(M3D, n, ch), iv(M3D, n, ch), m)
                else:
                    if (n, ch) == (0, 0):
                        for nn in range(IMG_PER_CORE):
                            vts[nn] = vpool.tile(
                                [128, 4 * VL], FP16, tag="v", name=f"vt_{kg}_{nn}"
                            )
                    a_, b_ = iv(AA, n, ch), iv(BB, n, ch)
                    c_, d_ = iv(M1C, n, ch), iv(M3D, n, ch)

                    def vv(vi):
                        o = vi * VL + ch * CHUNK
                        return vts[n][:, o : o + CHUNK]

                    nc.gpsimd.tensor_add(vv(0), iv(M0, n, ch), a_)
                    nc.gpsimd.tensor_add(vv(0), vv(0), c_)
                    nc.vector.scalar_tensor_tensor(vv(1), d_, 2.0, b_, A.mult, A.add)
                    nc.vector.scalar_tensor_tensor(vv(2), c_, 4.0, a_, A.mult, A.add)
                    nc.vector.scalar_tensor_tensor(vv(3), d_, 8.0, b_, A.mult, A.add)
                    nc.vector.tensor_add(vv(3), vv(3), m)
    for n in range(IMG_PER_CORE):
        ot = opool.tile([128, H * W], F32, tag="o", name=f"ot_{kg}_{n}")
        for vi in range(4):
            in_v = vts[n][:, vi * VL : vi * VL + UPL].rearrange(
                "p (hb w) -> p hb w", w=WP
            )[:, :, 1 : 1 + W]
            out_v = ot.rearrange("p (hb f w) -> p hb f w", f=4, w=W)[:, :, vi, :]
            nc.scalar.activation(
                out_v, in_v, mybir.ActivationFunctionType.Identity, bias=bsb[kg]
            )
        nc.sync.dma_start(out=y[n, kg], in_=ot[:])


def _build_tile_program(nc, loops, xp, wt, bi, y):
    with TileContext(nc) as tc:
        with (
            tc.tile_pool(name="wpool", bufs=1) as wpool,
            tc.tile_pool(name="xpool", bufs=1) as xpool,
            tc.tile_pool(name="upool", bufs=1) as upool,
            tc.tile_pool(name="tpool", bufs=1) as tpool,
            tc.tile_pool(name="ipool", bufs=1) as ipool,
            tc.tile_pool(name="vpool", bufs=3) as vpool,
            tc.tile_pool(name="opool", bufs=2) as opool,
            tc.tile_pool(name="pspool", bufs=8, space="PSUM") as pspool,
        ):
            wtile = wpool.tile([128, NW * 256], FP16, tag="w")
            nc.sync.dma_start(out=wtile[:], in_=wt[:])
            bsb = []
            for cg in range(2):
                btile = wpool.tile([128, 1], F32, tag=f"b{cg}")
                nc.sync.dma_start(out=btile[:], in_=bi[cg])
                bsb.append(btile)

            xsb, usb, tsb = {}, {}, {}
            for slot in range(2):
                for cg in range(2):
                    xsb[slot, cg] = xpool.tile(
                        [128, LEAD + L_PAD], FP16, tag=f"x{slot}{cg}", name=f"xsb{slot}{cg}"
                    )
                    nc.vector.memset(xsb[slot, cg][:], 0.0)
                    tsb[slot, cg] = tpool.tile([128, 2 * VL], FP16, tag=f"t{slot}{cg}", name=f"tsb{slot}{cg}")
            for n in range(IMG_PER_CORE):
                for cg in range(2):
                    usb[n, cg] = upool.tile([128, 6 * UL], FP16, tag=f"u{n}{cg}", name=f"usb{n}{cg}")
                    nc.vector.memset(usb[n, cg][:], 0.0)
            it = ipool.tile([128, 5 * 8 * 400], FP16, tag="i", name="it")

            import contextlib

            loop_cm = (
                tc.For_i(0, loops, 1, hint_engines=(mybir.EngineType.PE,))
                if loops > 1
                else contextlib.nullcontext()
            )
            with loop_cm:
                for n in range(IMG_PER_CORE):
                    slot = n % 2
                    for cg in range(2):
                        nc.sync.dma_start(
                            out=xsb[slot, cg][:, LEAD : LEAD + L_PAD], in_=xp[n, cg]
                        )
                    for cg in range(2):
                        _fwd_transform(nc, xsb[slot, cg], usb[n, cg], tsb[slot, cg])
                for kg in range(2):
                    _conv_all(nc, pspool, it, vpool, opool, usb, wtile, bsb, y, kg)


def _get_compiled(loops=1):
    key = (loops,)
    if key not in _compiled:
        _compiled[key] = _build_bass(loops)
    return _compiled[key]


def _prepare_inputs(x, weight, bias):
    x = np.asarray(x, dtype=np.float32)
    weight = np.asarray(weight, dtype=np.float32)
    bias = np.asarray(bias, dtype=np.float32)

    # padded pitch-57 fp16 activations, rows stored phase-major
    # (0,4,..,56 | 1,5,..,57 | 2,6,..,54 | 3,7,..,55)
    plane = np.zeros((N_IMG, C, HP, WP), dtype=np.float16)
    plane[:, :, 1 : 1 + H, 1 : 1 + W] = x.astype(np.float16)
    phase = np.concatenate(
        [plane[:, :, p::4, :] for p in range(4)], axis=2
    ).reshape(N_IMG, C, L_PLANE)
    xp = np.zeros((N_IMG, C, L_PAD), dtype=np.float16)
    xp[:, :, :L_PLANE] = phase

    # Winograd-transformed binarized weights:
    # wt[c', ((u*3+s)*2+cg)*256 + k] = sum_r G[u,r] sign(w)[k, cg*128+c', r, s]
    g = np.sign(weight)  # [K, C, 3, 3]
    Wt = np.einsum("ur,kcrs->ucsk", G_MAT, g.astype(np.float64))  # [6, C, 3, K]
    arr = Wt.reshape(6, 2, 128, 3, K).transpose(2, 0, 3, 1, 4)  # [128, 6, 3, 2, K]
    wt = np.ascontiguousarray(arr.reshape(128, NW * 256)).astype(np.float16)

    bi = bias.astype(np.float32).reshape(2, 128, 1)
    return xp, wt, bi


def kernel(x, weight, bias, _trace=False, _trace_kwargs=None):
    nc = _get_compiled()
    xp, wt, bi = _prepare_inputs(x, weight, bias)

    in_maps = []
    for i in range(N_CORES):
        xs = np.ascontiguousarray(
            xp[i * IMG_PER_CORE : (i + 1) * IMG_PER_CORE].reshape(
                IMG_PER_CORE, 2, 128, L_PAD
            )
        )
        in_maps.append({"xp": xs, "wt": wt, "bi": bi})

    res = run_bass_kernel_spmd(
        nc, in_maps, list(range(N_CORES)), trace=_trace, **(_trace_kwargs or {})
    )
    out = np.concatenate(
        [r["y"].reshape(IMG_PER_CORE, K, H, W) for r in res.results], axis=0
    )
    if _trace:
        return np.asarray(out, dtype=np.float32), res
    return np.asarray(out, dtype=np.float32)


# revision 10
# speedup vs baseline: 1.1568x; 1.1568x over previous
"""Binarized 3x3 conv (BinaryConnect) on 8 Trainium2 NeuronCores.

Problem: y = conv2d(x, sign(w), stride=1, pad=1) + bias
  x: (32, 256, 56, 56) f32, w: (256, 256, 3, 3) f32, bias: (256,) f32
  out: (32, 256, 56, 56) f32

Strategy (data-parallel over batch, 4 images/core), F(4,3) Winograd
along H + direct along W, all-fp16 datapath (same PE rate as bf16,
8x less quantization error, which the Winograd cancellations need):

  - Host: binarize weights, transform along r with G (F(4,3)),
    cast x to fp16, zero-pad each 56x56 plane into the pitch-57
    layout (shared pad column) used by the baseline.
  - Device, per image/channel-group: DVE+GpSimd compute the 6-point
    B^T forward transform along H into U planes [14 hb x 57] (one
    plane per Winograd point u; W-taps stay direct so every matmul
    rhs is a contiguous shift of a U plane). Per (kg, u, chunk):
    6 matmuls (3 s-taps x 2 cg) of 399 cols accumulate M_u in PSUM;
    the 6 u-points are processed in halves of 3 so at most 6+2 PSUM
    banks are live. Inverse A^T transform runs on DVE/GpSimd
    (scalar_tensor_tensor for the x2/x4/x8 terms), scalar engine
    applies bias + crops the pitch column + interleaves the 4
    Winograd output rows into packed 56x56 planes, DMA out.
  - PE work halves vs direct conv: 6 pts x 3 taps vs 9 taps x
    K=128-pairs => 229,824 vs 459,648 matmul columns per core.
"""

import numpy as np

import concourse.bacc as bacc
import concourse.mybir as mybir
from concourse.tile import TileContext
from concourse.bass_utils import run_bass_kernel_spmd

# problem constants (hardcoded per harness contract)
N_IMG = 32
C = 256  # input channels
K = 256  # output channels
H = W = 56
HP = 58  # padded rows (1 top + 56 + 1 bottom)
WP = 57  # row pitch: 1 shared pad column + 56 data
N_CORES = 8
IMG_PER_CORE = N_IMG // N_CORES

L_PLANE = HP * WP  # 3306
L_PAD = L_PLANE + 4  # 3310
LEAD = 2  # leading slack so tap offset (-1) stays in-bounds
X_SLACK = 188  # tail slack so the strided d_i views can be constructed
HB = 14  # h-blocks of 4 output rows
CHUNK = 7 * WP  # 399 cols per PSUM chunk (2 chunks per plane)
UPL = HB * WP  # 798: one U plane
U_LEAD = 2
UL = U_LEAD + UPL + 4  # 804 pitch between U planes
VL = 800  # pitch between v planes in the vstage tile
NW = 36  # distinct (u, s, cg) weight tiles

FP16 = mybir.dt.float16
F32 = mybir.dt.float32

# F(4,3) weight transform (Lavin / wincnn convention)
G_MAT = np.array(
    [
        [1 / 4, 0, 0],
        [-1 / 6, -1 / 6, -1 / 6],
        [-1 / 6, 1 / 6, -1 / 6],
        [1 / 24, 1 / 12, 1 / 6],
        [1 / 24, -1 / 12, 1 / 6],
        [0, 0, 1],
    ],
    np.float64,
)

_compiled = {}


def _ldw_key(inst):
    ap = inst.ins[0]
    bap = getattr(ap, "bass_ap", None)
    if bap is not None:
        try:
            return (bap.tensor.name, bap.offset, str(bap.ap), str(ap.dtype))
        except AttributeError:
            return None
    try:
        return (ap.memref, ap.offset, str(ap.ap), str(ap.dtype))
    except AttributeError:
        return None


def _dedup_ldweights(ordered):
    """Drop InstLdweights that reload weights already resident in the PE
    array. Only drops clean instructions: no sync_info and sync-deps
    covered by the retained load."""
    n_drop = 0
    for bb, insts in ordered.items():
        out = []
        last_key = None
        last_deps = None
        for inst in insts:
            if isinstance(inst, mybir.InstLdweights):
                key = _ldw_key(inst)
                si = inst.sync_info
                clean = si is None or (not si.on_wait and not si.on_update)
                sdeps = set(inst.sync_dependency_names())
                nsdeps = set(inst.nosync_dependency_names())
                if (
                    key is not None
                    and key == last_key
                    and clean
                    and last_deps is not None
                    and sdeps <= last_deps
                    and not nsdeps
                ):
                    n_drop += 1
                    continue
                last_key = key
                last_deps = sdeps
            elif isinstance(inst, mybir.InstMatmult):
                pass  # does not clobber the weight array
            elif getattr(inst, "engine", None) == mybir.EngineType.PE:
                last_key = None
            out.append(inst)
        ordered[bb] = out
    return n_drop


def _build_bass(loops=1):
    import concourse.tile as tile_mod

    nc = bacc.Bacc()

    xp = nc.declare_dram_parameter("xp", [IMG_PER_CORE, 2, 128, L_PAD], FP16, isOutput=False)
    wt = nc.declare_dram_parameter("wt", [128, NW * 256], FP16, isOutput=False)
    bi = nc.declare_dram_parameter("bi", [2, 128, 1], F32, isOutput=False)
    y = nc.declare_dram_parameter("y", [IMG_PER_CORE, 2, 128, H * W], F32, isOutput=True)

    orig_legalize = tile_mod.tile_legalize

    def legalize_and_dedup(ordered, nc_arg):
        ordered = orig_legalize(ordered, nc_arg)
        _dedup_ldweights(ordered)
        return ordered

    tile_mod.tile_legalize = legalize_and_dedup
    try:
        _build_tile_program(nc, loops, xp, wt, bi, y)
    finally:
        tile_mod.tile_legalize = orig_legalize
    nc.compile()
    return nc


# phase-major x layout: padded plane rows stored as [rows 0,4,..,56 |
# 1,5,..,57 | 2,6,..,54 | 3,7,..,55], so the rows {4*hb+i : hb} that feed
# Winograd point views are flat contiguous 798-col slices.
_PHASE_BASE = (0, 15 * WP, 30 * WP, 44 * WP)  # block starts (15,15,14,14 rows)
_DOFF = (  # offset of d_i = rows 4*hb+i, hb=0..13
    _PHASE_BASE[0],
    _PHASE_BASE[1],
    _PHASE_BASE[2],
    _PHASE_BASE[3],
    _PHASE_BASE[0] + WP,
    _PHASE_BASE[1] + WP,
)


def _dview(xt, i):
    """x rows 4*hb+i, both cg: [128, 2, 798] view (cg is dim 1)."""
    x3 = xt.rearrange("p (two l) -> p two l", two=2)
    return x3[:, :, LEAD + _DOFF[i] : LEAD + _DOFF[i] + UPL]


def _fwd_transform(nc, xt, ut, ts):
    """U_u = B^T d along H for both cg at once. DVE: 10 ops; GpSimd: 4."""
    A = mybir.AluOpType
    d = [_dview(xt, i) for i in range(6)]
    u3 = ut.rearrange("p (two l) -> p two l", two=2)
    U = [u3[:, :, u * UL + U_LEAD : u * UL + U_LEAD + UPL] for u in range(6)]
    t3 = ts.rearrange("p (two l) -> p two l", two=2)
    T0 = t3[:, :, 0:UPL]
    T1 = t3[:, :, VL : VL + UPL]
    v, g = nc.vector, nc.gpsimd
    # u0 = -5*d2 + (4*d0 + d4)
    v.scalar_tensor_tensor(T0, d[0], 4.0, d[4], A.mult, A.add)
    v.scalar_tensor_tensor(U[0], d[2], -5.0, T0, A.mult, A.add)
    # u1 = -4*(d1+d2) + (d3+d4)
    g.tensor_add(U[1], d[1], d[2])
    g.tensor_add(T1, d[3], d[4])
    v.scalar_tensor_tensor(U[1], U[1], -4.0, T1, A.mult, A.add)
    # u2 = 4*(d1-d2) + (d4-d3)
    g.tensor_sub(U[2], d[1], d[2])
    g.tensor_sub(T1, d[4], d[3])
    v.scalar_tensor_tensor(U[2], U[2], 4.0, T1, A.mult, A.add)
    # u3 = 2*(d3-d1) + (d4-d2);  u4 = -2*(d3-d1) + (d4-d2)
    v.tensor_sub(U[3], d[3], d[1])
    v.tensor_sub(T0, d[4], d[2])
    v.scalar_tensor_tensor(U[4], U[3], -2.0, T0, A.mult, A.add)
    v.scalar_tensor_tensor(U[3], U[3], 2.0, T0, A.mult, A.add)
    # u5 = -5*d3 + (4*d1 + d5)
    v.scalar_tensor_tensor(T0, d[1], 4.0, d[5], A.mult, A.add)
    v.scalar_tensor_tensor(U[5], d[3], -5.0, T0, A.mult, A.add)


def _conv_pair(nc, pspool, it, vpool, opool, usb, wtile, bsb, y, p, kg):
    """u-major GEMMs for one image pair + incremental inverse transform.

    Per u-step: 6 weight loads (s, cg) x 4 matmuls (2 img x 2 chunks);
    each image's two 399-col chunks land in one 2-bank PSUM tile at
    offsets 0 and 512, so every consumer op covers both chunks with one
    [2,399] strided AP. Incremental A^T inverse:
      u0: m0=M0   u1: m1=M1   u2: a=m1+M2, b=m1-M2   u3: m3=M3
      u4: c=m3+M4, d=m3-M4 (reusing m1/m3 slots)
      u5: v0=m0+a+c, v1=2d+b, v2=4c+a, v3=8d+b+M5
    """
    A = mybir.AluOpType
    M0, M1C, AA, BB, M3D = 0, 1, 2, 3, 4
    imgs = (2 * p, 2 * p + 1)

    def iv(j, ni):  # [128, 2(ch), 399] intermediate plane pair
        o = (j * 2 + ni) * 800
        return it[:, o : o + 800].rearrange("p (ch l) -> p ch l", l=400)[:, :, :CHUNK]

    vts = {}
    ps = {}
    for u in range(6):
        for ni in range(2):
            ps[ni] = pspool.tile(
                [128, 1024], F32, tag="ps", name=f"ps_{kg}_{u}_{p}_{ni}"
            )
        for si, (s, cgi) in enumerate((s, c) for s in range(3) for c in range(2)):
            col = ((u * 3 + s) * 2 + cgi) * 256 + kg * 128
            wsl = wtile[:, col : col + 128]
            for ni, n in enumerate(imgs):
                for ch in range(2):
                    base = cgi * (6 * UL) + u * UL + U_LEAD + ch * CHUNK + s - 1
                    nc.tensor.matmul(
                        ps[ni][:, ch * 512 : ch * 512 + CHUNK],
                        lhsT=wsl,
                        rhs=usb[n][:, base : base + CHUNK],
                        start=(si == 0),
                        stop=(si == 5),
                    )
        for ni, n in enumerate(imgs):
            m = ps[ni].rearrange("p (ch l) -> p ch l", l=512)[:, :, :CHUNK]
            if u == 0:
                nc.scalar.copy(iv(M0, ni), m)
            elif u == 1:
                nc.scalar.copy(iv(M1C, ni), m)
            elif u == 2:
                nc.vector.tensor_add(iv(AA, ni), iv(M1C, ni), m)
                nc.vector.tensor_sub(iv(BB, ni), iv(M1C, ni), m)
            elif u == 3:
                nc.scalar.copy(iv(M3D, ni), m)
            elif u == 4:
                nc.vector.tensor_add(iv(M1C, ni), iv(M3D, ni), m)
                nc.vector.tensor_sub(iv(M3D, ni), iv(M3D, ni), m)
            else:
                vt = vpool.tile([128, 4 * VL], FP16, tag="v", name=f"vt_{kg}_{n}")
                vts[n] = vt

                def vv(vi):
                    o = vi * VL
                    return vt[:, o : o + 2 * CHUNK].rearrange(
                        "p (ch l) -> p ch l", l=CHUNK
                    )

                a_, b_ = iv(AA, ni), iv(BB, ni)
                c_, d_ = iv(M1C, ni), iv(M3D, ni)
                nc.gpsimd.tensor_add(vv(0), iv(M0, ni), a_)
                nc.gpsimd.tensor_add(vv(0), vv(0), c_)
                nc.vector.scalar_tensor_tensor(vv(1), d_, 2.0, b_, A.mult, A.add)
                nc.vector.scalar_tensor_tensor(vv(2), c_, 4.0, a_, A.mult, A.add)
                nc.vector.scalar_tensor_tensor(vv(3), d_, 8.0, b_, A.mult, A.add)
                nc.vector.tensor_add(vv(3), vv(3), m)
    for n in imgs:
        ot = opool.tile([128, H * W], F32, tag="o", name=f"ot_{kg}_{n}")
        for vi in range(4):
            in_v = vts[n][:, vi * VL : vi * VL + UPL].rearrange(
                "p (hb w) -> p hb w", w=WP
            )[:, :, 1 : 1 + W]
            out_v = ot.rearrange("p (hb f w) -> p hb f w", f=4, w=W)[:, :, vi, :]
            nc.scalar.activation(
                out_v, in_v, mybir.ActivationFunctionType.Identity, bias=bsb[kg]
            )
        nc.sync.dma_start(out=y[n, kg], in_=ot[:])


def _build_tile_program(nc, loops, xp, wt, bi, y):
    XB = LEAD + L_PAD  # per-cg block inside a combined x tile

    with TileContext(nc) as tc:
        with (
            tc.tile_pool(name="wpool", bufs=1) as wpool,
            tc.tile_pool(name="xpool", bufs=1) as xpool,
            tc.tile_pool(name="upool", bufs=1) as upool,
            tc.tile_pool(name="tpool", bufs=1) as tpool,
            tc.tile_pool(name="ipool", bufs=2) as ipool,
            tc.tile_pool(name="vpool", bufs=3) as vpool,
            tc.tile_pool(name="opool", bufs=2) as opool,
            tc.tile_pool(name="pspool", bufs=4, space="PSUM") as pspool,
        ):
            wtile = wpool.tile([128, NW * 256], FP16, tag="w")
            nc.sync.dma_start(out=wtile[:], in_=wt[:])
            bsb = []
            for cg in range(2):
                btile = wpool.tile([128, 1], F32, tag=f"b{cg}")
                nc.sync.dma_start(out=btile[:], in_=bi[cg])
                bsb.append(btile)

            xsb, usb, tsb = {}, {}, {}
            for slot in range(2):
                xsb[slot] = xpool.tile(
                    [128, 2 * XB], FP16, tag=f"x{slot}", name=f"xsb{slot}"
                )
                nc.vector.memset(xsb[slot][:], 0.0)
                tsb[slot] = tpool.tile(
                    [128, 2 * 2 * VL], FP16, tag=f"t{slot}", name=f"tsb{slot}"
                )
            for n in range(IMG_PER_CORE):
                usb[n] = upool.tile(
                    [128, 2 * 6 * UL], FP16, tag=f"u{n}", name=f"usb{n}"
                )
                nc.vector.memset(usb[n][:], 0.0)

            import contextlib

            loop_cm = (
                tc.For_i(0, loops, 1, hint_engines=(mybir.EngineType.PE,))
                if loops > 1
                else contextlib.nullcontext()
            )
            with loop_cm:
                for p in range(2):
                    for n in (2 * p, 2 * p + 1):
                        slot = n % 2
                        for cg in range(2):
                            nc.sync.dma_start(
                                out=xsb[slot][:, cg * XB + LEAD : cg * XB + LEAD + L_PAD],
                                in_=xp[n, cg],
                            )
                        _fwd_transform(nc, xsb[slot], usb[n], tsb[slot])
                    for kg in range(2):
                        it = ipool.tile([128, 5 * 2 * 800], FP16, tag="i", name=f"it{p}{kg}")
                        _conv_pair(
                            nc, pspool, it, vpool, opool, usb, wtile, bsb, y, p, kg
                        )


def _get_compiled(loops=1):
    key = (loops,)
    if key not in _compiled:
        _compiled[key] = _build_bass(loops)
    return _compiled[key]


def _prepare_inputs(x, weight, bias):
    x = np.asarray(x, dtype=np.float32)
    weight = np.asarray(weight, dtype=np.float32)
    bias = np.asarray(bias, dtype=np.float32)

    # padded pitch-57 fp16 activations, rows stored phase-major
    # (0,4,..,56 | 1,5,..,57 | 2,6,..,54 | 3,7,..,55)
    plane = np.zeros((N_IMG, C, HP, WP), dtype=np.float16)
    plane[:, :, 1 : 1 + H, 1 : 1 + W] = x.astype(np.float16)
    phase = np.concatenate(
        [plane[:, :, p::4, :] for p in range(4)], axis=2
    ).reshape(N_IMG, C, L_PLANE)
    xp = np.zeros((N_IMG, C, L_PAD), dtype=np.float16)
    xp[:, :, :L_PLANE] = phase

    # Winograd-transformed binarized weights:
    # wt[c', ((u*3+s)*2+cg)*256 + k] = sum_r G[u,r] sign(w)[k, cg*128+c', r, s]
    g = np.sign(weight)  # [K, C, 3, 3]
    Wt = np.einsum("ur,kcrs->ucsk", G_MAT, g.astype(np.float64))  # [6, C, 3, K]
    arr = Wt.reshape(6, 2, 128, 3, K).transpose(2, 0, 3, 1, 4)  # [128, 6, 3, 2, K]
    wt = np.ascontiguousarray(arr.reshape(128, NW * 256)).astype(np.float16)

    bi = bias.astype(np.float32).reshape(2, 128, 1)
    return xp, wt, bi


def kernel(x, weight, bias, _trace=False, _trace_kwargs=None):
    nc = _get_compiled()
    xp, wt, bi = _prepare_inputs(x, weight, bias)

    in_maps = []
    for i in range(N_CORES):
        xs = np.ascontiguousarray(
            xp[i * IMG_PER_CORE : (i + 1) * IMG_PER_CORE].reshape(
                IMG_PER_CORE, 2, 128, L_PAD
            )
        )
        in_maps.append({"xp": xs, "wt": wt, "bi": bi})

    res = run_bass_kernel_spmd(
        nc, in_maps, list(range(N_CORES)), trace=_trace, **(_trace_kwargs or {})
    )
    out = np.concatenate(
        [r["y"].reshape(IMG_PER_CORE, K, H, W) for r in res.results], axis=0
    )
    if _trace:
        return np.asarray(out, dtype=np.float32), res
    return np.asarray(out, dtype=np.float32)


# revision 13
# speedup vs baseline: 1.2192x; 1.0540x over previous
"""Binarized 3x3 conv (BinaryConnect) on 8 Trainium2 NeuronCores.

Problem: y = conv2d(x, sign(w), stride=1, pad=1) + bias
  x: (32, 256, 56, 56) f32, w: (256, 256, 3, 3) f32, bias: (256,) f32
  out: (32, 256, 56, 56) f32

Strategy (data-parallel over batch, 4 images/core), F(4,3) Winograd
along H + direct along W, all-fp16 datapath (same PE rate as bf16,
8x less quantization error, which the Winograd cancellations need):

  - Host: binarize weights, transform along r with G (F(4,3)),
    cast x to fp16, zero-pad each 56x56 plane into the pitch-57
    layout (shared pad column) used by the baseline.
  - Device, per image/channel-group: DVE+GpSimd compute the 6-point
    B^T forward transform along H into U planes [14 hb x 57] (one
    plane per Winograd point u; W-taps stay direct so every matmul
    rhs is a contiguous shift of a U plane). Per (kg, u, chunk):
    6 matmuls (3 s-taps x 2 cg) of 399 cols accumulate M_u in PSUM;
    the 6 u-points are processed in halves of 3 so at most 6+2 PSUM
    banks are live. Inverse A^T transform runs on DVE/GpSimd
    (scalar_tensor_tensor for the x2/x4/x8 terms), scalar engine
    applies bias + crops the pitch column + interleaves the 4
    Winograd output rows into packed 56x56 planes, DMA out.
  - PE work halves vs direct conv: 6 pts x 3 taps vs 9 taps x
    K=128-pairs => 229,824 vs 459,648 matmul columns per core.
"""

import numpy as np

import concourse.bacc as bacc
import concourse.mybir as mybir
from concourse.tile import TileContext
from concourse.bass_utils import run_bass_kernel_spmd

# problem constants (hardcoded per harness contract)
N_IMG = 32
C = 256  # input channels
K = 256  # output channels
H = W = 56
HP = 58  # padded rows (1 top + 56 + 1 bottom)
WP = 57  # row pitch: 1 shared pad column + 56 data
N_CORES = 8
IMG_PER_CORE = N_IMG // N_CORES

L_PLANE = HP * WP  # 3306
L_PAD = L_PLANE + 4  # 3310
LEAD = 2  # leading slack so tap offset (-1) stays in-bounds
X_SLACK = 188  # tail slack so the strided d_i views can be constructed
HB = 28  # h-blocks of 2 output rows
CHUNK = 7 * WP  # 399 cols per PSUM chunk
UPL = 14 * WP  # 798: one U half-plane (14 hb)
U_LEAD = 2
UL = U_LEAD + UPL + 4  # 804 pitch between U (u, half) planes
VL = 2 * UPL + 8  # 1604: pitch between v planes (2 halves adjacent)
NW = 24  # distinct (u, s, cg) weight tiles

FP16 = mybir.dt.float16
F32 = mybir.dt.float32

# F(2,3) weight transform (Lavin / wincnn convention)
G_MAT = np.array(
    [
        [1, 0, 0],
        [1 / 2, 1 / 2, 1 / 2],
        [1 / 2, -1 / 2, 1 / 2],
        [0, 0, 1],
    ],
    np.float64,
)

_compiled = {}


def _ldw_key(inst):
    ap = inst.ins[0]
    bap = getattr(ap, "bass_ap", None)
    if bap is not None:
        try:
            return (bap.tensor.name, bap.offset, str(bap.ap), str(ap.dtype))
        except AttributeError:
            return None
    try:
        return (ap.memref, ap.offset, str(ap.ap), str(ap.dtype))
    except AttributeError:
        return None


def _dedup_ldweights(ordered):
    """Drop InstLdweights that reload weights already resident in the PE
    array. Only drops clean instructions: no sync_info and sync-deps
    covered by the retained load."""
    n_drop = 0
    for bb, insts in ordered.items():
        out = []
        last_key = None
        last_deps = None
        for inst in insts:
            if isinstance(inst, mybir.InstLdweights):
                key = _ldw_key(inst)
                si = inst.sync_info
                clean = si is None or (not si.on_wait and not si.on_update)
                sdeps = set(inst.sync_dependency_names())
                nsdeps = set(inst.nosync_dependency_names())
                if (
                    key is not None
                    and key == last_key
                    and clean
                    and last_deps is not None
                    and sdeps <= last_deps
                    and not nsdeps
                ):
                    n_drop += 1
                    continue
                last_key = key
                last_deps = sdeps
            elif isinstance(inst, mybir.InstMatmult):
                pass  # does not clobber the weight array
            elif getattr(inst, "engine", None) == mybir.EngineType.PE:
                last_key = None
            out.append(inst)
        ordered[bb] = out
    return n_drop


def _build_bass(loops=1):
    import concourse.tile as tile_mod

    nc = bacc.Bacc()

    xp = nc.declare_dram_parameter("xp", [IMG_PER_CORE, 2, 128, L_PAD], FP16, isOutput=False)
    wt = nc.declare_dram_parameter("wt", [128, NW * 256], FP16, isOutput=False)
    bi = nc.declare_dram_parameter("bi", [2, 128, 1], F32, isOutput=False)
    y = nc.declare_dram_parameter("y", [IMG_PER_CORE, 2, 128, H * W], F32, isOutput=True)

    orig_legalize = tile_mod.tile_legalize

    def legalize_and_dedup(ordered, nc_arg):
        ordered = orig_legalize(ordered, nc_arg)
        _dedup_ldweights(ordered)
        return ordered

    tile_mod.tile_legalize = legalize_and_dedup
    try:
        _build_tile_program(nc, loops, xp, wt, bi, y)
    finally:
        tile_mod.tile_legalize = orig_legalize
    nc.compile()
    return nc


# phase-major x layout (mod 2): padded plane rows stored as
# [rows 0,2,..,56 | rows 1,3,..,57], so rows {2*hb+i : hb} are flat slices.
_PHASE_BASE = (0, 29 * WP)
_DOFF = (  # offset of d_i = rows 2*hb+i
    _PHASE_BASE[0],
    _PHASE_BASE[1],
    _PHASE_BASE[0] + WP,
    _PHASE_BASE[1] + WP,
)


def _dview(xt, i, half):
    """x rows 2*hb+i (hb = 14*half..14*half+13), both cg: [128,2,798]."""
    x3 = xt.rearrange("p (two l) -> p two l", two=2)
    o = LEAD + _DOFF[i] + half * UPL
    return x3[:, :, o : o + UPL]


def _fwd_transform(nc, xt, ut):
    """U_u = B^T d along H (F(2,3)): u0=d0-d2, u1=d1+d2, u2=d2-d1, u3=d1-d3.
    Both cg at once; DVE and GpSimd split the 4 ops per half."""
    u3t = ut.rearrange("p (two l) -> p two l", two=2)
    for half in range(2):
        d = [_dview(xt, i, half) for i in range(4)]
        U = [
            u3t[:, :, (u * 2 + half) * UL + U_LEAD : (u * 2 + half) * UL + U_LEAD + UPL]
            for u in range(4)
        ]
        nc.vector.tensor_sub(U[0], d[0], d[2])
        nc.gpsimd.tensor_add(U[1], d[1], d[2])
        nc.vector.tensor_sub(U[2], d[2], d[1])
        nc.gpsimd.tensor_sub(U[3], d[1], d[3])


def _conv_img(nc, pspool, it, vpool, opool, usb, wtile, bsb, y, n, kg):
    """u-major GEMMs for one image + incremental F(2,3) inverse.

    Per u-step: 6 weight loads (s, cg) x 4 matmuls (2 half x 2 chunks);
    each half's two 399-col chunks land in one 2-bank PSUM tile.
    Incremental A^T inverse (v0 = M0+M1+M2, v1 = M1-M2-M3):
      u0: m0 = M0          u1: t0 = m0+M1, m1 = M1
      u2: v0 = t0+M2, t1 = m1-M2     u3: v1 = t1-M3
    """
    M0T, M1T = 0, 1  # it plane roles: (m0 -> t0), (m1 -> t1)

    def iv(j, half):
        o = (j * 2 + half) * 800
        return it[:, o : o + 800].rearrange("p (ch l) -> p ch l", l=400)[:, :, :CHUNK]

    vt = vpool.tile([128, 2 * VL], FP16, tag="v", name=f"vt_{kg}_{n}")

    def vv(vi, half):
        o = vi * VL + half * UPL
        return vt[:, o : o + 2 * CHUNK].rearrange("p (ch l) -> p ch l", l=CHUNK)

    ps = {}
    for u in range(4):
        for half in range(2):
            ps[half] = pspool.tile(
                [128, 1024], F32, tag="ps", name=f"ps_{kg}_{u}_{n}_{half}"
            )
        for si, (s, cgi) in enumerate((s, c) for s in range(3) for c in range(2)):
            col = ((u * 3 + s) * 2 + cgi) * 256 + kg * 128
            wsl = wtile[:, col : col + 128]
            for half in range(2):
                for ch in range(2):
                    base = (
                        cgi * (8 * UL)
                        + (u * 2 + half) * UL
                        + U_LEAD
                        + ch * CHUNK
                        + s
                        - 1
                    )
                    nc.tensor.matmul(
                        ps[half][:, ch * 512 : ch * 512 + CHUNK],
                        lhsT=wsl,
                        rhs=usb[n][:, base : base + CHUNK],
                        start=(si == 0),
                        stop=(si == 5),
                    )
        for half in range(2):
            m = ps[half].rearrange("p (ch l) -> p ch l", l=512)[:, :, :CHUNK]
            if u == 0:
                nc.scalar.copy(iv(M0T, half), m)
            elif u == 1:
                nc.vector.tensor_add(iv(M0T, half), iv(M0T, half), m)
                nc.scalar.copy(iv(M1T, half), m)
            elif u == 2:
                nc.vector.tensor_add(vv(0, half), iv(M0T, half), m)
                nc.vector.tensor_sub(iv(M1T, half), iv(M1T, half), m)
            else:
                nc.vector.tensor_sub(vv(1, half), iv(M1T, half), m)
    ot = opool.tile([128, H * W], F32, tag="o", name=f"ot_{kg}_{n}")
    for vi in range(2):
        in_v = vt[:, vi * VL : vi * VL + 2 * UPL].rearrange(
            "p (hb w) -> p hb w", w=WP
        )[:, :, 1 : 1 + W]
        out_v = ot.rearrange("p (hb f w) -> p hb f w", f=2, w=W)[:, :, vi, :]
        nc.scalar.activation(
            out_v, in_v, mybir.ActivationFunctionType.Identity, bias=bsb[kg]
        )
    nc.sync.dma_start(out=y[n, kg], in_=ot[:])


def _build_tile_program(nc, loops, xp, wt, bi, y):
    XB = LEAD + L_PAD  # per-cg block inside a combined x tile

    with TileContext(nc) as tc:
        with (
            tc.tile_pool(name="wpool", bufs=1) as wpool,
            tc.tile_pool(name="xpool", bufs=1) as xpool,
            tc.tile_pool(name="upool", bufs=1) as upool,
            tc.tile_pool(name="ipool", bufs=2) as ipool,
            tc.tile_pool(name="vpool", bufs=3) as vpool,
            tc.tile_pool(name="opool", bufs=2) as opool,
            tc.tile_pool(name="pspool", bufs=4, space="PSUM") as pspool,
        ):
            wtile = wpool.tile([128, NW * 256], FP16, tag="w")
            nc.sync.dma_start(out=wtile[:], in_=wt[:])
            bsb = []
            for cg in range(2):
                btile = wpool.tile([128, 1], F32, tag=f"b{cg}")
                nc.sync.dma_start(out=btile[:], in_=bi[cg])
                bsb.append(btile)

            xsb, usb = {}, {}
            for slot in range(2):
                xsb[slot] = xpool.tile(
                    [128, 2 * XB], FP16, tag=f"x{slot}", name=f"xsb{slot}"
                )
                nc.vector.memset(xsb[slot][:], 0.0)
            for n in range(IMG_PER_CORE):
                usb[n] = upool.tile(
                    [128, 2 * 8 * UL], FP16, tag=f"u{n}", name=f"usb{n}"
                )
                nc.vector.memset(usb[n][:], 0.0)

            import contextlib

            loop_cm = (
                tc.For_i(0, loops, 1, hint_engines=(mybir.EngineType.PE,))
                if loops > 1
                else contextlib.nullcontext()
            )
            with loop_cm:
                for p in range(2):
                    for n in (2 * p, 2 * p + 1):
                        slot = n % 2
                        for cg in range(2):
                            nc.sync.dma_start(
                                out=xsb[slot][:, cg * XB + LEAD : cg * XB + LEAD + L_PAD],
                                in_=xp[n, cg],
                            )
                        _fwd_transform(nc, xsb[slot], usb[n])
                    for kg in range(2):
                        for n in (2 * p, 2 * p + 1):
                            it = ipool.tile(
                                [128, 2 * 2 * 800], FP16, tag="i", name=f"it{p}{kg}{n}"
                            )
                            _conv_img(
                                nc, pspool, it, vpool, opool, usb, wtile, bsb, y, n, kg
                            )


def _get_compiled(loops=1):
    key = (loops,)
    if key not in _compiled:
        _compiled[key] = _build_bass(loops)
    return _compiled[key]


def _prepare_inputs(x, weight, bias):
    x = np.asarray(x, dtype=np.float32)
    weight = np.asarray(weight, dtype=np.float32)
    bias = np.asarray(bias, dtype=np.float32)

    # padded pitch-57 fp16 activations, rows stored phase-major
    # (0,4,..,56 | 1,5,..,57 | 2,6,..,54 | 3,7,..,55)
    plane = np.zeros((N_IMG, C, HP, WP), dtype=np.float16)
    plane[:, :, 1 : 1 + H, 1 : 1 + W] = x.astype(np.float16)
    phase = np.concatenate(
        [plane[:, :, p::2, :] for p in range(2)], axis=2
    ).reshape(N_IMG, C, L_PLANE)
    xp = np.zeros((N_IMG, C, L_PAD), dtype=np.float16)
    xp[:, :, :L_PLANE] = phase

    # Winograd-transformed binarized weights:
    # wt[c', ((u*3+s)*2+cg)*256 + k] = sum_r G[u,r] sign(w)[k, cg*128+c', r, s]
    g = np.sign(weight)  # [K, C, 3, 3]
    Wt = np.einsum("ur,kcrs->ucsk", G_MAT, g.astype(np.float64))  # [6, C, 3, K]
    arr = Wt.reshape(4, 2, 128, 3, K).transpose(2, 0, 3, 1, 4)  # [128, 4, 3, 2, K]
    wt = np.ascontiguousarray(arr.reshape(128, NW * 256)).astype(np.float16)

    bi = bias.astype(np.float32).reshape(2, 128, 1)
    return xp, wt, bi


def kernel(x, weight, bias, _trace=False, _trace_kwargs=None):
    nc = _get_compiled()
    xp, wt, bi = _prepare_inputs(x, weight, bias)

    in_maps = []
    for i in range(N_CORES):
        xs = np.ascontiguousarray(
            xp[i * IMG_PER_CORE : (i + 1) * IMG_PER_CORE].reshape(
                IMG_PER_CORE, 2, 128, L_PAD
            )
        )
        in_maps.append({"xp": xs, "wt": wt, "bi": bi})

    res = run_bass_kernel_spmd(
        nc, in_maps, list(range(N_CORES)), trace=_trace, **(_trace_kwargs or {})
    )
    out = np.concatenate(
        [r["y"].reshape(IMG_PER_CORE, K, H, W) for r in res.results], axis=0
    )
    if _trace:
        return np.asarray(out, dtype=np.float32), res
    return np.asarray(out, dtype=np.float32)


# revision 15
# speedup vs baseline: 1.3854x; 1.1363x over previous
"""Binarized 3x3 conv (BinaryConnect) on 8 Trainium2 NeuronCores.

Problem: y = conv2d(x, sign(w), stride=1, pad=1) + bias
  x: (32, 256, 56, 56) f32, w: (256, 256, 3, 3) f32, bias: (256,) f32
  out: (32, 256, 56, 56) f32

Strategy (data-parallel over batch, 4 images/core), F(4,3) Winograd
along H + direct along W, all-fp16 datapath (same PE rate as bf16,
8x less quantization error, which the Winograd cancellations need):

  - Host: binarize weights, transform along r with G (F(4,3)),
    cast x to fp16, zero-pad each 56x56 plane into the pitch-57
    layout (shared pad column) used by the baseline.
  - Device, per image/channel-group: DVE+GpSimd compute the 6-point
    B^T forward transform along H into U planes [14 hb x 57] (one
    plane per Winograd point u; W-taps stay direct so every matmul
    rhs is a contiguous shift of a U plane). Per (kg, u, chunk):
    6 matmuls (3 s-taps x 2 cg) of 399 cols accumulate M_u in PSUM;
    the 6 u-points are processed in halves of 3 so at most 6+2 PSUM
    banks are live. Inverse A^T transform runs on DVE/GpSimd
    (scalar_tensor_tensor for the x2/x4/x8 terms), scalar engine
    applies bias + crops the pitch column + interleaves the 4
    Winograd output rows into packed 56x56 planes, DMA out.
  - PE work halves vs direct conv: 6 pts x 3 taps vs 9 taps x
    K=128-pairs => 229,824 vs 459,648 matmul columns per core.
"""

import numpy as np

import concourse.bacc as bacc
import concourse.mybir as mybir
from concourse.tile import TileContext
from concourse.bass_utils import run_bass_kernel_spmd

# problem constants (hardcoded per harness contract)
N_IMG = 32
C = 256  # input channels
K = 256  # output channels
H = W = 56
HP = 58  # padded rows (1 top + 56 + 1 bottom)
WP = 57  # row pitch: 1 shared pad column + 56 data
N_CORES = 8
IMG_PER_CORE = N_IMG // N_CORES

L_PLANE = HP * WP  # 3306
L_PAD = L_PLANE + 4  # 3310
LEAD = 2  # leading slack so tap offset (-1) stays in-bounds
X_SLACK = 188  # tail slack so the strided d_i views can be constructed
HB = 28  # h-blocks of 2 output rows
CHUNK = 7 * WP  # 399 cols per PSUM chunk
UPL = 14 * WP  # 798: one U half-plane (14 hb)
U_LEAD = 2
UL = U_LEAD + UPL + 4  # 804 pitch between U (u, half) planes
VL = 2 * UPL + 8  # 1604: pitch between v planes (2 halves adjacent)
NW = 24  # distinct (u, s, cg) weight tiles

FP16 = mybir.dt.float16
F32 = mybir.dt.float32

# F(2,3) weight transform (Lavin / wincnn convention)
G_MAT = np.array(
    [
        [1, 0, 0],
        [1 / 2, 1 / 2, 1 / 2],
        [1 / 2, -1 / 2, 1 / 2],
        [0, 0, 1],
    ],
    np.float64,
)

_compiled = {}


def _ldw_key(inst):
    ap = inst.ins[0]
    bap = getattr(ap, "bass_ap", None)
    if bap is not None:
        try:
            return (bap.tensor.name, bap.offset, str(bap.ap), str(ap.dtype))
        except AttributeError:
            return None
    try:
        return (ap.memref, ap.offset, str(ap.ap), str(ap.dtype))
    except AttributeError:
        return None


def _dedup_ldweights(ordered):
    """Drop InstLdweights that reload weights already resident in the PE
    array. Only drops clean instructions: no sync_info and sync-deps
    covered by the retained load."""
    n_drop = 0
    for bb, insts in ordered.items():
        out = []
        last_key = None
        last_deps = None
        for inst in insts:
            if isinstance(inst, mybir.InstLdweights):
                key = _ldw_key(inst)
                si = inst.sync_info
                clean = si is None or (not si.on_wait and not si.on_update)
                sdeps = set(inst.sync_dependency_names())
                nsdeps = set(inst.nosync_dependency_names())
                if (
                    key is not None
                    and key == last_key
                    and clean
                    and last_deps is not None
                    and sdeps <= last_deps
                    and not nsdeps
                ):
                    n_drop += 1
                    continue
                last_key = key
                last_deps = sdeps
            elif isinstance(inst, mybir.InstMatmult):
                pass  # does not clobber the weight array
            elif getattr(inst, "engine", None) == mybir.EngineType.PE:
                last_key = None
            out.append(inst)
        ordered[bb] = out
    return n_drop


def _build_bass(loops=1):
    import concourse.tile as tile_mod

    nc = bacc.Bacc()

    xp = nc.declare_dram_parameter("xp", [IMG_PER_CORE, 2, 128, L_PAD], FP16, isOutput=False)
    wt = nc.declare_dram_parameter("wt", [128, NW * 256], FP16, isOutput=False)
    bi = nc.declare_dram_parameter("bi", [2, 128, 1], F32, isOutput=False)
    y = nc.declare_dram_parameter("y", [IMG_PER_CORE, 2, 128, H * W], F32, isOutput=True)

    orig_legalize = tile_mod.tile_legalize

    def legalize_and_dedup(ordered, nc_arg):
        ordered = orig_legalize(ordered, nc_arg)
        _dedup_ldweights(ordered)
        return ordered

    tile_mod.tile_legalize = legalize_and_dedup
    try:
        _build_tile_program(nc, loops, xp, wt, bi, y)
    finally:
        tile_mod.tile_legalize = orig_legalize
    nc.compile()
    return nc


# phase-major x layout (mod 2): padded plane rows stored as
# [rows 0,2,..,56 | rows 1,3,..,57], so rows {2*hb+i : hb} are flat slices.
_PHASE_BASE = (0, 29 * WP)
_DOFF = (  # offset of d_i = rows 2*hb+i
    _PHASE_BASE[0],
    _PHASE_BASE[1],
    _PHASE_BASE[0] + WP,
    _PHASE_BASE[1] + WP,
)


def _dview(xt, i, half):
    """x rows 2*hb+i (hb = 14*half..14*half+13), both cg: [128,2,798]."""
    x3 = xt.rearrange("p (two l) -> p two l", two=2)
    o = LEAD + _DOFF[i] + half * UPL
    return x3[:, :, o : o + UPL]


def _fwd_transform(nc, xt, ut):
    """U_u = B^T d along H (F(2,3)): u0=d0-d2, u1=d1+d2, u2=d2-d1, u3=d1-d3.
    Both cg at once; DVE and GpSimd split the 4 ops per half."""
    u3t = ut.rearrange("p (two l) -> p two l", two=2)
    for half in range(2):
        d = [_dview(xt, i, half) for i in range(4)]
        U = [
            u3t[:, :, (u * 2 + half) * UL + U_LEAD : (u * 2 + half) * UL + U_LEAD + UPL]
            for u in range(4)
        ]
        nc.vector.tensor_sub(U[0], d[0], d[2])
        nc.gpsimd.tensor_add(U[1], d[1], d[2])
        nc.vector.tensor_sub(U[2], d[2], d[1])
        nc.gpsimd.tensor_sub(U[3], d[1], d[3])


def _conv_img(nc, pspool, it, vpool, opool, usb, wtile, bsb, y, n, kg):
    """u-major GEMMs for one image + incremental F(2,3) inverse.

    Per u-step: 6 weight loads (s, cg) x 4 matmuls (2 half x 2 chunks);
    each half's two 399-col chunks land in one 2-bank PSUM tile.
    Incremental A^T inverse (v0 = M0+M1+M2, v1 = M1-M2-M3):
      u0: m0 = M0          u1: t0 = m0+M1, m1 = M1
      u2: v0 = t0+M2, t1 = m1-M2     u3: v1 = t1-M3
    """
    M0T, M1T = 0, 1  # it plane roles: (m0 -> t0), (m1 -> t1)

    def iv(j, half):
        o = (j * 2 + half) * 800
        return it[:, o : o + 800].rearrange("p (ch l) -> p ch l", l=400)[:, :, :CHUNK]

    vt = vpool.tile([128, 2 * VL], FP16, tag="v", name=f"vt_{kg}_{n}")

    def vv(vi, half):
        o = vi * VL + half * UPL
        return vt[:, o : o + 2 * CHUNK].rearrange("p (ch l) -> p ch l", l=CHUNK)

    ps = {}
    for u in range(4):
        for half in range(2):
            ps[half] = pspool.tile(
                [128, 1024], F32, tag="ps", name=f"ps_{kg}_{u}_{n}_{half}"
            )
        for si, (s, cgi) in enumerate((s, c) for s in range(3) for c in range(2)):
            col = ((u * 3 + s) * 2 + cgi) * 256 + kg * 128
            wsl = wtile[:, col : col + 128]
            for half in range(2):
                for ch in range(2):
                    base = (
                        cgi * (8 * UL)
                        + (u * 2 + half) * UL
                        + U_LEAD
                        + ch * CHUNK
                        + s
                        - 1
                    )
                    nc.tensor.matmul(
                        ps[half][:, ch * 512 : ch * 512 + CHUNK],
                        lhsT=wsl,
                        rhs=usb[n][:, base : base + CHUNK],
                        start=(si == 0),
                        stop=(si == 5),
                    )
        for half in range(2):
            m = ps[half].rearrange("p (ch l) -> p ch l", l=512)[:, :, :CHUNK]
            if u == 0:
                nc.scalar.copy(iv(M0T, half), m)
            elif u == 1:
                nc.vector.tensor_add(iv(M0T, half), iv(M0T, half), m)
                nc.scalar.copy(iv(M1T, half), m)
            elif u == 2:
                nc.vector.tensor_add(vv(0, half), iv(M0T, half), m)
                nc.vector.tensor_sub(iv(M1T, half), iv(M1T, half), m)
            else:
                nc.vector.tensor_sub(vv(1, half), iv(M1T, half), m)
    ot = opool.tile([128, H * W], F32, tag="o", name=f"ot_{kg}_{n}")
    for vi in range(2):
        in_v = vt[:, vi * VL : vi * VL + 2 * UPL].rearrange(
            "p (hb w) -> p hb w", w=WP
        )[:, :, 1 : 1 + W]
        out_v = ot.rearrange("p (hb f w) -> p hb f w", f=2, w=W)[:, :, vi, :]
        nc.scalar.activation(
            out_v, in_v, mybir.ActivationFunctionType.Identity, bias=bsb[kg]
        )
    nc.sync.dma_start(out=y[n, kg], in_=ot[:])


def _build_tile_program(nc, loops, xp, wt, bi, y):
    XB = LEAD + L_PAD  # per-cg block inside a combined x tile

    with TileContext(nc) as tc:
        with (
            tc.tile_pool(name="wpool", bufs=1) as wpool,
            tc.tile_pool(name="xpool", bufs=1) as xpool,
            tc.tile_pool(name="upool", bufs=1) as upool,
            tc.tile_pool(name="ipool", bufs=2) as ipool,
            tc.tile_pool(name="vpool", bufs=3) as vpool,
            tc.tile_pool(name="opool", bufs=2) as opool,
            tc.tile_pool(name="pspool", bufs=4, space="PSUM") as pspool,
        ):
            wtile = wpool.tile([128, NW * 256], FP16, tag="w")
            nc.sync.dma_start(out=wtile[:], in_=wt[:])
            bsb = []
            for cg in range(2):
                btile = wpool.tile([128, 1], F32, tag=f"b{cg}")
                nc.sync.dma_start(out=btile[:], in_=bi[cg])
                bsb.append(btile)

            xsb, usb = {}, {}
            for slot in range(2):
                xsb[slot] = xpool.tile(
                    [128, 2 * XB], FP16, tag=f"x{slot}", name=f"xsb{slot}"
                )
            for n in range(IMG_PER_CORE):
                usb[n] = upool.tile(
                    [128, 2 * 8 * UL], FP16, tag=f"u{n}", name=f"usb{n}"
                )
                # only the inter-plane gap columns need zeroing (the DMA
                # and forward transform overwrite all data columns)
                upl = usb[n].rearrange("p (pl l) -> p pl l", l=UL)
                nc.vector.memset(upl[:, :, 0:U_LEAD], 0.0)
                nc.vector.memset(upl[:, :, U_LEAD + UPL : UL], 0.0)

            import contextlib

            loop_cm = (
                tc.For_i(0, loops, 1, hint_engines=(mybir.EngineType.PE,))
                if loops > 1
                else contextlib.nullcontext()
            )
            def load_and_fwd(n):
                slot = n % 2
                for cg in range(2):
                    nc.sync.dma_start(
                        out=xsb[slot][:, cg * XB + LEAD : cg * XB + LEAD + L_PAD],
                        in_=xp[n, cg],
                    )
                _fwd_transform(nc, xsb[slot], usb[n])

            def gemm_pair(p):
                for kg in range(2):
                    for n in (2 * p, 2 * p + 1):
                        it = ipool.tile(
                            [128, 2 * 2 * 800], FP16, tag="i", name=f"it{p}{kg}{n}"
                        )
                        _conv_img(
                            nc, pspool, it, vpool, opool, usb, wtile, bsb, y, n, kg
                        )

            # software pipeline, rotated by one image pair: the body
            # transforms pair 1 first (overlapping pair-0 GEMMs, which read
            # U produced by the previous trip / prologue), then re-transforms
            # pair 0 for the next trip while pair-1 GEMMs run.
            for n in (0, 1):  # prologue
                load_and_fwd(n)
            with loop_cm:
                for n in (2, 3):
                    load_and_fwd(n)
                gemm_pair(0)
                for n in (0, 1):
                    load_and_fwd(n)
                gemm_pair(1)


def _get_compiled(loops=1):
    key = (loops,)
    if key not in _compiled:
        _compiled[key] = _build_bass(loops)
    return _compiled[key]


def _prepare_inputs(x, weight, bias):
    x = np.asarray(x, dtype=np.float32)
    weight = np.asarray(weight, dtype=np.float32)
    bias = np.asarray(bias, dtype=np.float32)

    # padded pitch-57 fp16 activations, rows stored phase-major
    # (0,4,..,56 | 1,5,..,57 | 2,6,..,54 | 3,7,..,55)
    plane = np.zeros((N_IMG, C, HP, WP), dtype=np.float16)
    plane[:, :, 1 : 1 + H, 1 : 1 + W] = x.astype(np.float16)
    phase = np.concatenate(
        [plane[:, :, p::2, :] for p in range(2)], axis=2
    ).reshape(N_IMG, C, L_PLANE)
    xp = np.zeros((N_IMG, C, L_PAD), dtype=np.float16)
    xp[:, :, :L_PLANE] = phase

    # Winograd-transformed binarized weights:
    # wt[c', ((u*3+s)*2+cg)*256 + k] = sum_r G[u,r] sign(w)[k, cg*128+c', r, s]
    g = np.sign(weight)  # [K, C, 3, 3]
    Wt = np.einsum("ur,kcrs->ucsk", G_MAT, g.astype(np.float64))  # [6, C, 3, K]
    arr = Wt.reshape(4, 2, 128, 3, K).transpose(2, 0, 3, 1, 4)  # [128, 4, 3, 2, K]
    wt = np.ascontiguousarray(arr.reshape(128, NW * 256)).astype(np.float16)

    bi = bias.astype(np.float32).reshape(2, 128, 1)
    return xp, wt, bi


def kernel(x, weight, bias, _trace=False, _trace_kwargs=None):
    nc = _get_compiled()
    xp, wt, bi = _prepare_inputs(x, weight, bias)

    in_maps = []
    for i in range(N_CORES):
        xs = np.ascontiguousarray(
            xp[i * IMG_PER_CORE : (i + 1) * IMG_PER_CORE].reshape(
                IMG_PER_CORE, 2, 128, L_PAD
            )
        )
        in_maps.append({"xp": xs, "wt": wt, "bi": bi})

    res = run_bass_kernel_spmd(
        nc, in_maps, list(range(N_CORES)), trace=_trace, **(_trace_kwargs or {})
    )
    out = np.concatenate(
        [r["y"].reshape(IMG_PER_CORE, K, H, W) for r in res.results], axis=0
    )
    if _trace:
        return np.asarray(out, dtype=np.float32), res
    return np.asarray(out, dtype=np.float32)


# revision 16
# speedup vs baseline: 1.4108x; 1.0183x over previous
"""Binarized 3x3 conv (BinaryConnect) on 8 Trainium2 NeuronCores.

Problem: y = conv2d(x, sign(w), stride=1, pad=1) + bias
  x: (32, 256, 56, 56) f32, w: (256, 256, 3, 3) f32, bias: (256,) f32
  out: (32, 256, 56, 56) f32

Strategy (data-parallel over batch, 4 images/core), F(4,3) Winograd
along H + direct along W, all-fp16 datapath (same PE rate as bf16,
8x less quantization error, which the Winograd cancellations need):

  - Host: binarize weights, transform along r with G (F(4,3)),
    cast x to fp16, zero-pad each 56x56 plane into the pitch-57
    layout (shared pad column) used by the baseline.
  - Device, per image/channel-group: DVE+GpSimd compute the 6-point
    B^T forward transform along H into U planes [14 hb x 57] (one
    plane per Winograd point u; W-taps stay direct so every matmul
    rhs is a contiguous shift of a U plane). Per (kg, u, chunk):
    6 matmuls (3 s-taps x 2 cg) of 399 cols accumulate M_u in PSUM;
    the 6 u-points are processed in halves of 3 so at most 6+2 PSUM
    banks are live. Inverse A^T transform runs on DVE/GpSimd
    (scalar_tensor_tensor for the x2/x4/x8 terms), scalar engine
    applies bias + crops the pitch column + interleaves the 4
    Winograd output rows into packed 56x56 planes, DMA out.
  - PE work halves vs direct conv: 6 pts x 3 taps vs 9 taps x
    K=128-pairs => 229,824 vs 459,648 matmul columns per core.
"""

import numpy as np

import concourse.bacc as bacc
import concourse.mybir as mybir
from concourse.tile import TileContext
from concourse.bass_utils import run_bass_kernel_spmd

# problem constants (hardcoded per harness contract)
N_IMG = 32
C = 256  # input channels
K = 256  # output channels
H = W = 56
HP = 58  # padded rows (1 top + 56 + 1 bottom)
WP = 57  # row pitch: 1 shared pad column + 56 data
N_CORES = 8
IMG_PER_CORE = N_IMG // N_CORES

L_PLANE = HP * WP  # 3306
L_PAD = L_PLANE + 4  # 3310
LEAD = 2  # leading slack so tap offset (-1) stays in-bounds
X_SLACK = 188  # tail slack so the strided d_i views can be constructed
HB = 28  # h-blocks of 2 output rows
CHUNK = 7 * WP  # 399 cols per PSUM chunk
UPL = 14 * WP  # 798: one U half-plane (14 hb)
U_LEAD = 2
UL = U_LEAD + UPL + 4  # 804 pitch between U (u, half) planes
VL = 2 * UPL + 8  # 1604: pitch between v planes (2 halves adjacent)
NW = 24  # distinct (u, s, cg) weight tiles

FP16 = mybir.dt.float16
F32 = mybir.dt.float32

# F(2,3) weight transform (Lavin / wincnn convention)
G_MAT = np.array(
    [
        [1, 0, 0],
        [1 / 2, 1 / 2, 1 / 2],
        [1 / 2, -1 / 2, 1 / 2],
        [0, 0, 1],
    ],
    np.float64,
)

_compiled = {}


def _ldw_key(inst):
    ap = inst.ins[0]
    bap = getattr(ap, "bass_ap", None)
    if bap is not None:
        try:
            return (bap.tensor.name, bap.offset, str(bap.ap), str(ap.dtype))
        except AttributeError:
            return None
    try:
        return (ap.memref, ap.offset, str(ap.ap), str(ap.dtype))
    except AttributeError:
        return None


def _dedup_ldweights(ordered):
    """Drop InstLdweights that reload weights already resident in the PE
    array. Only drops clean instructions: no sync_info and sync-deps
    covered by the retained load."""
    n_drop = 0
    for bb, insts in ordered.items():
        out = []
        last_key = None
        last_deps = None
        for inst in insts:
            if isinstance(inst, mybir.InstLdweights):
                key = _ldw_key(inst)
                si = inst.sync_info
                clean = si is None or (not si.on_wait and not si.on_update)
                sdeps = set(inst.sync_dependency_names())
                nsdeps = set(inst.nosync_dependency_names())
                if (
                    key is not None
                    and key == last_key
                    and clean
                    and last_deps is not None
                    and sdeps <= last_deps
                    and not nsdeps
                ):
                    n_drop += 1
                    continue
                last_key = key
                last_deps = sdeps
            elif isinstance(inst, mybir.InstMatmult):
                pass  # does not clobber the weight array
            elif getattr(inst, "engine", None) == mybir.EngineType.PE:
                last_key = None
            out.append(inst)
        ordered[bb] = out
    return n_drop


def _build_bass(loops=1):
    import concourse.tile as tile_mod

    nc = bacc.Bacc()

    xp = nc.declare_dram_parameter("xp", [IMG_PER_CORE, 2, 128, L_PAD], FP16, isOutput=False)
    wt = nc.declare_dram_parameter("wt", [128, NW * 256], FP16, isOutput=False)
    bi = nc.declare_dram_parameter("bi", [2, 128, 1], F32, isOutput=False)
    y = nc.declare_dram_parameter("y", [IMG_PER_CORE, 2, 128, H * W], F32, isOutput=True)

    orig_legalize = tile_mod.tile_legalize

    def legalize_and_dedup(ordered, nc_arg):
        ordered = orig_legalize(ordered, nc_arg)
        _dedup_ldweights(ordered)
        return ordered

    tile_mod.tile_legalize = legalize_and_dedup
    try:
        _build_tile_program(nc, loops, xp, wt, bi, y)
    finally:
        tile_mod.tile_legalize = orig_legalize
    nc.compile()
    return nc


# phase-major x layout (mod 2): padded plane rows stored as
# [rows 0,2,..,56 | rows 1,3,..,57], so rows {2*hb+i : hb} are flat slices.
_PHASE_BASE = (0, 29 * WP)
_DOFF = (  # offset of d_i = rows 2*hb+i
    _PHASE_BASE[0],
    _PHASE_BASE[1],
    _PHASE_BASE[0] + WP,
    _PHASE_BASE[1] + WP,
)


def _dview(xt, i, half):
    """x rows 2*hb+i (hb = 14*half..14*half+13), both cg: [128,2,798]."""
    x3 = xt.rearrange("p (two l) -> p two l", two=2)
    o = LEAD + _DOFF[i] + half * UPL
    return x3[:, :, o : o + UPL]


def _fwd_transform(nc, xt, ut):
    """U_u = B^T d along H (F(2,3)): u0=d0-d2, u1=d1+d2, u2=d2-d1, u3=d1-d3.
    One op per point covering both cg and both halves (rows {2hb+i} are
    contiguous across halves in the phase-major layout); GpSimd takes only
    u2 so the slower engine is never the last writer by much."""
    x3 = xt.rearrange("p (two l) -> p two l", two=2)
    u4t = ut.rearrange("p (two pl l) -> p two pl l", two=2, l=UL)

    def dv(i):
        o = LEAD + _DOFF[i]
        return x3[:, :, o : o + 2 * UPL].rearrange("p two (h l) -> p two h l", l=UPL)

    d = [dv(i) for i in range(4)]
    U = [
        u4t[:, :, 2 * u : 2 * u + 2, U_LEAD : U_LEAD + UPL]
        for u in range(4)
    ]
    nc.vector.tensor_sub(U[0], d[0], d[2])
    nc.vector.tensor_add(U[1], d[1], d[2])
    nc.gpsimd.tensor_sub(U[2], d[2], d[1])
    nc.vector.tensor_sub(U[3], d[1], d[3])


def _conv_img(nc, pspool, it, vpool, opool, usb, wtile, bsb, y, n, kg):
    """u-major GEMMs for one image + incremental F(2,3) inverse.

    Per u-step: 6 weight loads (s, cg) x 4 matmuls (2 half x 2 chunks);
    each half's two 399-col chunks land in one 2-bank PSUM tile.
    Incremental A^T inverse (v0 = M0+M1+M2, v1 = M1-M2-M3):
      u0: m0 = M0          u1: t0 = m0+M1, m1 = M1
      u2: v0 = t0+M2, t1 = m1-M2     u3: v1 = t1-M3
    """
    M0T, M1T = 0, 1  # it plane roles: (m0 -> t0), (m1 -> t1)

    def iv(j, half):
        o = (j * 2 + half) * 800
        return it[:, o : o + 800].rearrange("p (ch l) -> p ch l", l=400)[:, :, :CHUNK]

    vt = vpool.tile([128, 2 * VL], FP16, tag="v", name=f"vt_{kg}_{n}")

    def vv(vi, half):
        o = vi * VL + half * UPL
        return vt[:, o : o + 2 * CHUNK].rearrange("p (ch l) -> p ch l", l=CHUNK)

    ps = {}
    for u in range(4):
        for half in range(2):
            ps[half] = pspool.tile(
                [128, 1024], F32, tag="ps", name=f"ps_{kg}_{u}_{n}_{half}"
            )
        for si, (s, cgi) in enumerate((s, c) for s in range(3) for c in range(2)):
            col = ((u * 3 + s) * 2 + cgi) * 256 + kg * 128
            wsl = wtile[:, col : col + 128]
            for half in range(2):
                for ch in range(2):
                    base = (
                        cgi * (8 * UL)
                        + (u * 2 + half) * UL
                        + U_LEAD
                        + ch * CHUNK
                        + s
                        - 1
                    )
                    nc.tensor.matmul(
                        ps[half][:, ch * 512 : ch * 512 + CHUNK],
                        lhsT=wsl,
                        rhs=usb[n][:, base : base + CHUNK],
                        start=(si == 0),
                        stop=(si == 5),
                    )
        for half in range(2):
            m = ps[half].rearrange("p (ch l) -> p ch l", l=512)[:, :, :CHUNK]
            if u == 0:
                nc.scalar.copy(iv(M0T, half), m)
            elif u == 1:
                nc.vector.tensor_add(iv(M0T, half), iv(M0T, half), m)
                nc.scalar.copy(iv(M1T, half), m)
            elif u == 2:
                nc.vector.tensor_add(vv(0, half), iv(M0T, half), m)
                nc.vector.tensor_sub(iv(M1T, half), iv(M1T, half), m)
            else:
                nc.vector.tensor_sub(vv(1, half), iv(M1T, half), m)
    ot = opool.tile([128, H * W], F32, tag="o", name=f"ot_{kg}_{n}")
    for vi in range(2):
        in_v = vt[:, vi * VL : vi * VL + 2 * UPL].rearrange(
            "p (hb w) -> p hb w", w=WP
        )[:, :, 1 : 1 + W]
        out_v = ot.rearrange("p (hb f w) -> p hb f w", f=2, w=W)[:, :, vi, :]
        nc.scalar.activation(
            out_v, in_v, mybir.ActivationFunctionType.Identity, bias=bsb[kg]
        )
    nc.sync.dma_start(out=y[n, kg], in_=ot[:])


def _build_tile_program(nc, loops, xp, wt, bi, y):
    XB = LEAD + L_PAD  # per-cg block inside a combined x tile

    with TileContext(nc) as tc:
        with (
            tc.tile_pool(name="wpool", bufs=1) as wpool,
            tc.tile_pool(name="xpool", bufs=1) as xpool,
            tc.tile_pool(name="upool", bufs=1) as upool,
            tc.tile_pool(name="ipool", bufs=2) as ipool,
            tc.tile_pool(name="vpool", bufs=3) as vpool,
            tc.tile_pool(name="opool", bufs=2) as opool,
            tc.tile_pool(name="pspool", bufs=4, space="PSUM") as pspool,
        ):
            wtile = wpool.tile([128, NW * 256], FP16, tag="w")
            nc.sync.dma_start(out=wtile[:], in_=wt[:])
            bsb = []
            for cg in range(2):
                btile = wpool.tile([128, 1], F32, tag=f"b{cg}")
                nc.sync.dma_start(out=btile[:], in_=bi[cg])
                bsb.append(btile)

            xsb, usb = {}, {}
            for slot in range(2):
                xsb[slot] = xpool.tile(
                    [128, 2 * XB], FP16, tag=f"x{slot}", name=f"xsb{slot}"
                )
            for n in range(IMG_PER_CORE):
                usb[n] = upool.tile(
                    [128, 2 * 8 * UL], FP16, tag=f"u{n}", name=f"usb{n}"
                )
                # only the inter-plane gap columns need zeroing (the DMA
                # and forward transform overwrite all data columns)
                upl = usb[n].rearrange("p (pl l) -> p pl l", l=UL)
                nc.vector.memset(upl[:, :, 0:U_LEAD], 0.0)
                nc.vector.memset(upl[:, :, U_LEAD + UPL : UL], 0.0)

            import contextlib

            loop_cm = (
                tc.For_i(0, loops, 1, hint_engines=(mybir.EngineType.PE,))
                if loops > 1
                else contextlib.nullcontext()
            )
            def load_and_fwd(n):
                slot = n % 2
                for cg in range(2):
                    nc.sync.dma_start(
                        out=xsb[slot][:, cg * XB + LEAD : cg * XB + LEAD + L_PAD],
                        in_=xp[n, cg],
                    )
                _fwd_transform(nc, xsb[slot], usb[n])

            def gemm_pair(p):
                for kg in range(2):
                    for n in (2 * p, 2 * p + 1):
                        it = ipool.tile(
                            [128, 2 * 2 * 800], FP16, tag="i", name=f"it{p}{kg}{n}"
                        )
                        _conv_img(
                            nc, pspool, it, vpool, opool, usb, wtile, bsb, y, n, kg
                        )

            # software pipeline, rotated by one image pair: the body
            # transforms pair 1 first (overlapping pair-0 GEMMs, which read
            # U produced by the previous trip / prologue), then re-transforms
            # pair 0 for the next trip while pair-1 GEMMs run.
            for n in (0, 1):  # prologue
                load_and_fwd(n)
            with loop_cm:
                for n in (2, 3):
                    load_and_fwd(n)
                gemm_pair(0)
                for n in (0, 1):
                    load_and_fwd(n)
                gemm_pair(1)


def _get_compiled(loops=1):
    key = (loops,)
    if key not in _compiled:
        _compiled[key] = _build_bass(loops)
    return _compiled[key]


def _prepare_inputs(x, weight, bias):
    x = np.asarray(x, dtype=np.float32)
    weight = np.asarray(weight, dtype=np.float32)
    bias = np.asarray(bias, dtype=np.float32)

    # padded pitch-57 fp16 activations, rows stored phase-major
    # (0,4,..,56 | 1,5,..,57 | 2,6,..,54 | 3,7,..,55)
    plane = np.zeros((N_IMG, C, HP, WP), dtype=np.float16)
    plane[:, :, 1 : 1 + H, 1 : 1 + W] = x.astype(np.float16)
    phase = np.concatenate(
        [plane[:, :, p::2, :] for p in range(2)], axis=2
    ).reshape(N_IMG, C, L_PLANE)
    xp = np.zeros((N_IMG, C, L_PAD), dtype=np.float16)
    xp[:, :, :L_PLANE] = phase

    # Winograd-transformed binarized weights:
    # wt[c', ((u*3+s)*2+cg)*256 + k] = sum_r G[u,r] sign(w)[k, cg*128+c', r, s]
    g = np.sign(weight)  # [K, C, 3, 3]
    Wt = np.einsum("ur,kcrs->ucsk", G_MAT, g.astype(np.float64))  # [6, C, 3, K]
    arr = Wt.reshape(4, 2, 128, 3, K).transpose(2, 0, 3, 1, 4)  # [128, 4, 3, 2, K]
    wt = np.ascontiguousarray(arr.reshape(128, NW * 256)).astype(np.float16)

    bi = bias.astype(np.float32).reshape(2, 128, 1)
    return xp, wt, bi


def kernel(x, weight, bias, _trace=False, _trace_kwargs=None):
    nc = _get_compiled()
    xp, wt, bi = _prepare_inputs(x, weight, bias)

    in_maps = []
    for i in range(N_CORES):
        xs = np.ascontiguousarray(
            xp[i * IMG_PER_CORE : (i + 1) * IMG_PER_CORE].reshape(
                IMG_PER_CORE, 2, 128, L_PAD
            )
        )
        in_maps.append({"xp": xs, "wt": wt, "bi": bi})

    res = run_bass_kernel_spmd(
        nc, in_maps, list(range(N_CORES)), trace=_trace, **(_trace_kwargs or {})
    )
    out = np.concatenate(
        [r["y"].reshape(IMG_PER_CORE, K, H, W) for r in res.results], axis=0
    )
    if _trace:
        return np.asarray(out, dtype=np.float32), res
    return np.asarray(out, dtype=np.float32)
